# revision 6
# baseline (speedup 1.0000x reference)
"""Multi-head attention (B=2, S=2048, D=1024, H=16, d_k=64) on 8 trn2 cores.

Sharding: batch (2) x head-groups (4 groups of 4 heads). Each core computes
its batch's full sequence for its 4 heads plus the partial output projection
(w_o row-sharded); host sums the 4 partials per batch and adds b_o.

Device-side layout trick: all inputs are pre-transposed on the host to
[d_model, seq] so every matmul contracts over the partition dimension:
  qT/kT = w_c @ x.T            [256, 2048]  (d on partitions)
  vh    = x @ w_c.T            [2048, 256]  (natural; + ones column)
  scoresT[t, s] = kT_h.T-block @ qT_h  (K=64, two heads packed in the PE
                                        array via tile_position row groups)
  attnT = exp(scoresT / 8)     (no max subtraction: scores ~ N(0,1))
  av    = [vh_h | 1].T @ attnT (M=65: row 64 accumulates the softmax
                                denominators for free)
  out   = (av/denom).T @ w_oT  (partial over this core's 256 dims)
"""

import numpy as np

P = 128
S = 2048
DM = 1024
DH = 256          # head dims per core (4 heads x 64)
H = 4             # heads per core
DK = 64
MC = DM // P      # 8 m-chunks
TC = S // P       # 16 t-chunks
ST = 1024         # s-tile for scores/exp psum tiles
NST = S // ST     # 2
N_CORES = 8

_COMPILED = None


def _build():
    import concourse.bacc as bacc
    import concourse.mybir as mybir
    from concourse.tile import TileContext

    F32 = mybir.dt.float32
    AF = mybir.ActivationFunctionType
    OP = mybir.AluOpType

    nc = bacc.Bacc(None, target_bir_lowering=False)

    xq = nc.dram_tensor("xq", [DM, S], F32, kind="ExternalInput")
    xk = nc.dram_tensor("xk", [DM, S], F32, kind="ExternalInput")
    xv = nc.dram_tensor("xv", [DM, S], F32, kind="ExternalInput")
    wq = nc.dram_tensor("wq", [DM, DH], F32, kind="ExternalInput")
    wk = nc.dram_tensor("wk", [DM, DH], F32, kind="ExternalInput")
    wv = nc.dram_tensor("wv", [DM, DH], F32, kind="ExternalInput")
    bq = nc.dram_tensor("bq", [DH], F32, kind="ExternalInput")
    bk = nc.dram_tensor("bk", [DH], F32, kind="ExternalInput")
    bv = nc.dram_tensor("bv", [DH], F32, kind="ExternalInput")
    wo = nc.dram_tensor("wo", [DH, DM], F32, kind="ExternalInput")
    out = nc.dram_tensor("out", [S, DM], F32, kind="ExternalOutput")

    with TileContext(nc) as tc:
        with (
            tc.tile_pool(name="persist", bufs=1) as pp,
            tc.tile_pool(name="xfull", bufs=9) as xp,
            tc.tile_pool(name="wstream", bufs=10) as wp,
            tc.tile_pool(name="small", bufs=2) as sp,
            tc.tile_pool(name="ps_sc", bufs=2, space="PSUM") as ps_sc,
            tc.tile_pool(name="ps_av", bufs=2, space="PSUM") as ps_av,
        ):
            qT = pp.tile([P, 2, S], F32, name="qT")
            kT = pp.tile([P, 2, S], F32, name="kT")
            vh = pp.tile([P, TC, H, DK + 1], F32, name="vh")
            wo_sb = pp.tile([P, 2, DM], F32, name="wo_sb")
            o2a = pp.tile([P, S], F32, name="o2a")    # heads 0,1 (normalized)
            o2b = pp.tile([P, S], F32, name="o2b")    # heads 2,3
            ones = pp.tile([P, DK], F32, name="ones")
            bq_sb = pp.tile([P, 2], F32, name="bq_sb")
            bk_sb = pp.tile([P, 2], F32, name="bk_sb")
            bv_bc = pp.tile([P, DH], F32, name="bv_bc")

            nc.vector.memset(ones[:], 1.0)
            nc.vector.memset(vh[:, :, :, DK : DK + 1], 1.0)
            nc.sync.dma_start(bq_sb[:], bq[:].rearrange("(c p) -> p c", p=P))
            nc.sync.dma_start(bk_sb[:], bk[:].rearrange("(c p) -> p c", p=P))
            nc.sync.dma_start(bv_bc[:], bv[None, :].to_broadcast((P, DH)))
            nc.sync.dma_start(wo_sb[:], wo[:].rearrange("(c p) n -> p c n", p=P))

            # ---------------- Phase A: projections ----------------
            if True:
                # kT / qT: [d' on partitions, s free], k-contiguous per tile
                for name, xd, wd, b_sb, dstT in (
                    ("k", xk, wk, bk_sb, kT),
                    ("q", xq, wq, bq_sb, qT),
                ):
                    xcs = []
                    wcs = []
                    for mc in range(MC):
                        xc = xp.tile([P, S], F32, name="xc")
                        nc.sync.dma_start(xc[:], xd[mc * P : (mc + 1) * P, :])
                        wc = wp.tile([P, DH], F32, name="wc")
                        nc.sync.dma_start(wc[:], wd[mc * P : (mc + 1) * P, :])
                        xcs.append(xc)
                        wcs.append(wc)
                    for dc in range(2):
                        for st4 in range(4):
                            pool = (ps_sc, ps_av)[st4 % 2]
                            ps = pool.tile([P, 512], F32, name=("sc", "av")[st4 % 2])
                            for mc in range(MC):
                                nc.tensor.matmul(
                                    ps[:],
                                    wcs[mc][:, dc * P : (dc + 1) * P],
                                    xcs[mc][:, st4 * 512 : (st4 + 1) * 512],
                                    start=(mc == 0),
                                    stop=(mc == MC - 1),
                                )
                            nc.vector.tensor_scalar(
                                out=dstT[:, dc, st4 * 512 : (st4 + 1) * 512],
                                in0=ps[:],
                                scalar1=b_sb[:, dc : dc + 1],
                                scalar2=None,
                                op0=OP.add,
                            )

                # vh: natural [t, d'], + bias broadcast
                xcs = []
                wcs = []
                for mc in range(MC):
                    xc = xp.tile([P, S], F32, name="xc")
                    nc.sync.dma_start(xc[:], xv[mc * P : (mc + 1) * P, :])
                    wc = wp.tile([P, DH], F32, name="wc")
                    nc.sync.dma_start(wc[:], wv[mc * P : (mc + 1) * P, :])
                    xcs.append(xc)
                    wcs.append(wc)
                for tcc in range(TC):
                    pool = (ps_sc, ps_av)[tcc % 2]
                    ps = pool.tile([P, 512], F32, name=("sc", "av")[tcc % 2])
                    for mc in range(MC):
                        nc.tensor.matmul(
                            ps[:, 0:DH],
                            xcs[mc][:, tcc * P : (tcc + 1) * P],
                            wcs[mc][:],
                            start=(mc == 0),
                            stop=(mc == MC - 1),
                        )
                    nc.vector.tensor_tensor(
                        out=vh[:, tcc, :, 0:DK],
                        in0=ps[:, 0:DH].rearrange("p (h d) -> p h d", h=H),
                        in1=bv_bc[:].rearrange("p (h d) -> p h d", h=H),
                        op=OP.add,
                    )

            # ---------------- Phase B: attention ----------------
            if True:
                for pair in range(2):
                    o2 = (o2a, o2b)[pair]
                    for st2 in range(NST):
                        s0 = st2 * ST
                        avs = [ps_av.tile([P, ST], F32, name="av") for _ in range(2)]
                        for tcc in range(TC):
                            for hi in range(2):
                                sc = ps_sc.tile([P, ST], F32, name="sc")
                                lhs = kT[
                                    DK * hi : DK * (hi + 1),
                                    pair,
                                    tcc * P : (tcc + 1) * P,
                                ]
                                for hf in range(2):
                                    nc.tensor.matmul(
                                        sc[:, hf * 512 : (hf + 1) * 512],
                                        lhs,
                                        qT[
                                            DK * hi : DK * (hi + 1),
                                            pair,
                                            s0 + hf * 512 : s0 + (hf + 1) * 512,
                                        ],
                                        start=True,
                                        stop=True,
                                        tile_position=(DK * hi, 0),
                                    )
                                at = xp.tile([P, ST], F32, name="xc")
                                nc.scalar.activation(at[:], sc[:], AF.Exp, scale=0.125)
                                h = 2 * pair + hi
                                for hf in range(2):
                                    nc.tensor.matmul(
                                        avs[hi][0 : DK + 1, hf * 512 : (hf + 1) * 512],
                                        vh[:, tcc, h, :],
                                        at[:, hf * 512 : (hf + 1) * 512],
                                        start=(tcc == 0),
                                        stop=(tcc == TC - 1),
                                    )
                        # normalize: denom row 64 -> dsb row h; bcast via K=1 mm
                        for hi in range(2):
                            h = 2 * pair + hi
                            av = avs[hi]
                            dsb = sp.tile([1, ST], F32, name="dsb")
                            nc.vector.tensor_copy(dsb[0:1, :], av[DK : DK + 1, :])
                            rb_ps = ps_sc.tile([P, ST], F32, name="sc")
                            for hf in range(2):
                                nc.tensor.matmul(
                                    rb_ps[0:DK, hf * 512 : (hf + 1) * 512],
                                    ones[0:1, :],
                                    dsb[0:1, hf * 512 : (hf + 1) * 512],
                                    start=True,
                                    stop=True,
                                )
                            rb = sp.tile([DK, ST], F32, name="rb")
                            nc.vector.reciprocal(rb[:], rb_ps[0:DK, :])
                            nc.vector.tensor_mul(
                                o2[DK * hi : DK * (hi + 1), s0 : s0 + ST],
                                av[0:DK, :],
                                rb[:],
                            )

                # ---------------- Phase C: output projection ----------------
                for st7 in range(TC):
                    of_ps = ps_av.tile([P, ST], F32, name="av")
                    for c in range(2):
                        o2 = (o2a, o2b)[c]
                        for nh in range(2):
                            nc.tensor.matmul(
                                of_ps[:, nh * 512 : (nh + 1) * 512],
                                o2[:, st7 * P : (st7 + 1) * P],
                                wo_sb[:, c, nh * 512 : (nh + 1) * 512],
                                start=(c == 0),
                                stop=(c == 1),
                            )
                    of = xp.tile([P, ST], F32, name="xc")
                    nc.vector.tensor_copy(of[:], of_ps[:])
                    nc.sync.dma_start(out[st7 * P : (st7 + 1) * P, :], of[:])

    nc.compile()
    return nc


def _get_nc():
    global _COMPILED
    if _COMPILED is None:
        _COMPILED = _build()
    return _COMPILED


def _make_in_maps(q, k, v, w_q, b_q, w_k, b_k, w_v, b_v, w_o, b_o):
    q = np.asarray(q, np.float32)
    k = np.asarray(k, np.float32)
    v = np.asarray(v, np.float32)
    xqT = [np.ascontiguousarray(q[b].T) for b in range(2)]
    xkT = [np.ascontiguousarray(k[b].T) for b in range(2)]
    xvT = [np.ascontiguousarray(v[b].T) for b in range(2)]
    in_maps = []
    for core in range(N_CORES):
        b, hg = divmod(core, 4)
        sl = slice(hg * DH, (hg + 1) * DH)
        in_maps.append(
            {
                "xq": xqT[b],
                "xk": xkT[b],
                "xv": xvT[b],
                "wq": np.ascontiguousarray(np.asarray(w_q, np.float32)[sl, :].T),
                "wk": np.ascontiguousarray(np.asarray(w_k, np.float32)[sl, :].T),
                "wv": np.ascontiguousarray(np.asarray(w_v, np.float32)[sl, :].T),
                "bq": np.ascontiguousarray(np.asarray(b_q, np.float32)[sl]),
                "bk": np.ascontiguousarray(np.asarray(b_k, np.float32)[sl]),
                "bv": np.ascontiguousarray(np.asarray(b_v, np.float32)[sl]),
                "wo": np.ascontiguousarray(np.asarray(w_o, np.float32)[:, sl].T),
            }
        )
    return in_maps


def run(inputs, trace=False):
    from concourse.bass_utils import run_bass_kernel_spmd

    nc = _get_nc()
    in_maps = _make_in_maps(**inputs)
    res = run_bass_kernel_spmd(
        nc, in_maps, core_ids=list(range(N_CORES)), trace=trace
    )
    b_o = np.asarray(inputs["b_o"], np.float32)
    full = np.empty((2, S, DM), np.float32)
    for b in range(2):
        acc = res.results[4 * b]["out"].astype(np.float32)
        for hg in range(1, 4):
            acc = acc + res.results[4 * b + hg]["out"]
        full[b] = acc + b_o[None, :]
    return full, res


def kernel(**inputs) -> np.ndarray:
    full, _ = run(inputs, trace=False)
    return full


# revision 9
# speedup vs baseline: 1.2371x; 1.2371x over previous
"""Multi-head attention (B=2, S=2048, D=1024, H=16, d_k=64) on 8 trn2 cores.

Sharding: batch (2) x head-groups (4 groups of 4 heads). Each core computes
its batch's full sequence for its 4 heads plus the partial output projection
(w_o row-sharded); host sums the 4 partials per batch and adds b_o.

Numerics: fp32 PE matmuls on trn2 lower to 2-pass LOW_HIGH at half clock
(~4x slower than bf16). Projections and scores therefore use split-bf16
(x = hi + lo, 3-term hi*hi + hi*lo + lo*hi, fp32 PSUM accumulation,
~2^-18 per-product error). The attention*V matmul, softmax denominators
(a ones-column in the stationary), normalization, and output projection
stay fp32, keeping end-to-end error at fp32 grade (~1e-5).

Layout: all inputs host-pre-transposed to [d_model, seq] so every matmul
contracts along partitions:
  qT/kT  = w_c @ x.T  -> [256, 2048] (hi/lo bf16)
  vh     = x @ w_c.T  -> [2048, 256] fp32 natural (+ ones column)
  scoresT[t, s] (K=64, head pairs packed via tile_position row groups)
  attnT  = exp(scoresT/8)  (no max subtraction: scores ~ N(0,1))
  av     = [vh_h | 1].T @ attnT   (fp32, M=65: row 64 = denominators)
  out   += (av * bcast(1/denom)).T @ w_oT   (fp32 partial)
"""

import numpy as np

P = 128
S = 2048
DM = 1024
DH = 256          # head dims per core (4 heads x 64)
H = 4             # heads per core
DK = 64
MC = DM // P      # 8 m-chunks
TC = S // P       # 16 t-chunks
ST = 1024         # s-tile for scores/exp psum tiles
NST = S // ST     # 2
N_CORES = 8

_COMPILED = None


def _build():
    import concourse.bacc as bacc
    import concourse.mybir as mybir
    from concourse.tile import TileContext

    F32 = mybir.dt.float32
    BF16 = mybir.dt.bfloat16
    AF = mybir.ActivationFunctionType
    OP = mybir.AluOpType

    nc = bacc.Bacc(None, target_bir_lowering=False)

    xin = {}
    win = {}
    for t in ("q", "k", "v"):
        for p in ("h", "l"):
            xin[t + p] = nc.dram_tensor(f"x{t}{p}", [DM, S], BF16, kind="ExternalInput")
            win[t + p] = nc.dram_tensor(f"w{t}{p}", [DM, DH], BF16, kind="ExternalInput")
    bq = nc.dram_tensor("bq", [DH], F32, kind="ExternalInput")
    bk = nc.dram_tensor("bk", [DH], F32, kind="ExternalInput")
    bv = nc.dram_tensor("bv", [DH], F32, kind="ExternalInput")
    wo = nc.dram_tensor("wo", [DH, DM], F32, kind="ExternalInput")
    out = nc.dram_tensor("out", [S, DM], F32, kind="ExternalOutput")

    with TileContext(nc) as tc:
        with (
            tc.tile_pool(name="persist", bufs=1) as pp,
            tc.tile_pool(name="xfull", bufs=18) as xp,
            tc.tile_pool(name="wstream", bufs=20) as wp,
            tc.tile_pool(name="small", bufs=3) as sp,
            tc.tile_pool(name="ps_sc", bufs=2, space="PSUM") as ps_sc,
            tc.tile_pool(name="ps_av", bufs=2, space="PSUM") as ps_av,
        ):
            qTh = pp.tile([P, 2, S], BF16, name="qTh")
            qTl = pp.tile([P, 2, S], BF16, name="qTl")
            kTh = pp.tile([P, 2, S], BF16, name="kTh")
            kTl = pp.tile([P, 2, S], BF16, name="kTl")
            vh = pp.tile([P, TC, H, DK + 1], F32, name="vh")
            wo_sb = pp.tile([P, 2, DM], F32, name="wo_sb")
            o2a = pp.tile([P, S], F32, name="o2a")    # heads 0,1 (normalized)
            o2b = pp.tile([P, S], F32, name="o2b")    # heads 2,3
            ones = pp.tile([P, DK], F32, name="ones")
            bq_sb = pp.tile([P, 2], F32, name="bq_sb")
            bk_sb = pp.tile([P, 2], F32, name="bk_sb")
            bv_bc = pp.tile([P, DH], F32, name="bv_bc")

            nc.vector.memset(ones[:], 1.0)
            nc.vector.memset(vh[:, :, :, DK : DK + 1], 1.0)
            nc.sync.dma_start(bq_sb[:], bq[:].rearrange("(c p) -> p c", p=P))
            nc.sync.dma_start(bk_sb[:], bk[:].rearrange("(c p) -> p c", p=P))
            nc.sync.dma_start(bv_bc[:], bv[None, :].to_broadcast((P, DH)))
            nc.sync.dma_start(wo_sb[:], wo[:].rearrange("(c p) n -> p c n", p=P))

            # ---------------- Phase A: projections (split-bf16) -------------
            def load_chunks(xd_h, xd_l, wd_h, wd_l):
                xs, ws = [], []
                for mc in range(MC):
                    xh = xp.tile([P, S], BF16, name="xc")
                    xl = xp.tile([P, S], BF16, name="xc")
                    nc.sync.dma_start(xh[:], xd_h[mc * P : (mc + 1) * P, :])
                    nc.sync.dma_start(xl[:], xd_l[mc * P : (mc + 1) * P, :])
                    wh = wp.tile([P, DH], BF16, name="wc")
                    wl = wp.tile([P, DH], BF16, name="wc")
                    nc.sync.dma_start(wh[:], wd_h[mc * P : (mc + 1) * P, :])
                    nc.sync.dma_start(wl[:], wd_l[mc * P : (mc + 1) * P, :])
                    xs.append((xh, xl))
                    ws.append((wh, wl))
                return xs, ws

            # kT / qT: [d' on partitions, s free], k-contiguous per tile
            for t, b_sb, dTh, dTl in (("k", bk_sb, kTh, kTl), ("q", bq_sb, qTh, qTl)):
                xs, ws = load_chunks(xin[t + "h"], xin[t + "l"], win[t + "h"], win[t + "l"])
                for dc in range(2):
                    for st4 in range(4):
                        pool = (ps_sc, ps_av)[st4 % 2]
                        ps = pool.tile([P, 512], F32, name=("sc", "av")[st4 % 2])
                        nmm = 3 * MC
                        i = 0
                        for mc in range(MC):
                            xh, xl = xs[mc]
                            wh, wl = ws[mc]
                            for lt, rt in ((wh, xh), (wh, xl), (wl, xh)):
                                nc.tensor.matmul(
                                    ps[:],
                                    lt[:, dc * P : (dc + 1) * P],
                                    rt[:, st4 * 512 : (st4 + 1) * 512],
                                    start=(i == 0),
                                    stop=(i == nmm - 1),
                                )
                                i += 1
                        tmp = sp.tile([P, 512], F32, name="tmp")
                        sl = (slice(None), dc, slice(st4 * 512, (st4 + 1) * 512))
                        nc.vector.tensor_scalar(
                            out=tmp[:], in0=ps[:], scalar1=b_sb[:, dc : dc + 1],
                            scalar2=None, op0=OP.add,
                        )
                        nc.vector.tensor_copy(dTh[sl], tmp[:])
                        nc.vector.tensor_tensor(
                            out=dTl[sl], in0=tmp[:], in1=dTh[sl], op=OP.subtract
                        )

            # vh: natural [t, d'] fp32, + bias broadcast
            xs, ws = load_chunks(xin["vh"], xin["vl"], win["vh"], win["vl"])
            for tcc in range(TC):
                pool = (ps_sc, ps_av)[tcc % 2]
                ps = pool.tile([P, 512], F32, name=("sc", "av")[tcc % 2])
                nmm = 3 * MC
                i = 0
                for mc in range(MC):
                    xh, xl = xs[mc]
                    wh, wl = ws[mc]
                    for lt, rt in ((xh, wh), (xh, wl), (xl, wh)):
                        nc.tensor.matmul(
                            ps[:, 0:DH],
                            lt[:, tcc * P : (tcc + 1) * P],
                            rt[:],
                            start=(i == 0),
                            stop=(i == nmm - 1),
                        )
                        i += 1
                nc.vector.tensor_tensor(
                    out=vh[:, tcc, :, 0:DK],
                    in0=ps[:, 0:DH].rearrange("p (h d) -> p h d", h=H),
                    in1=bv_bc[:].rearrange("p (h d) -> p h d", h=H),
                    op=OP.add,
                )

            # ---------------- Phase B: attention ----------------
            for pair in range(2):
                o2 = (o2a, o2b)[pair]
                for st2 in range(NST):
                    s0 = st2 * ST
                    avs = [ps_av.tile([P, ST], F32, name="av") for _ in range(2)]
                    ats = {}

                    def scores_exp(tcc):
                        for hi2 in range(2):
                            rows = slice(DK * hi2, DK * (hi2 + 1))
                            sc = ps_sc.tile([P, ST], F32, name="sc")
                            for hf in range(2):
                                i = 0
                                for lt, rt in ((kTh, qTh), (kTh, qTl), (kTl, qTh)):
                                    nc.tensor.matmul(
                                        sc[:, hf * 512 : (hf + 1) * 512],
                                        lt[rows, pair, tcc * P : (tcc + 1) * P],
                                        rt[rows, pair, s0 + hf * 512 : s0 + (hf + 1) * 512],
                                        start=(i == 0),
                                        stop=(i == 2),
                                        tile_position=(DK * hi2, 0),
                                    )
                                    i += 1
                            at = xp.tile([P, ST], F32, name="xc")
                            nc.scalar.activation(at[:], sc[:], AF.Exp, scale=0.125)
                            ats[(tcc, hi2)] = at

                    def av_mm(tcc):
                        for hi2 in range(2):
                            at = ats.pop((tcc, hi2))
                            h = 2 * pair + hi2
                            for hf in range(2):
                                nc.tensor.matmul(
                                    avs[hi2][0 : DK + 1, hf * 512 : (hf + 1) * 512],
                                    vh[:, tcc, h, :],
                                    at[:, hf * 512 : (hf + 1) * 512],
                                    start=(tcc == 0),
                                    stop=(tcc == TC - 1),
                                )

                    # software pipeline: scores(tc+1) issues before av(tc)
                    scores_exp(0)
                    for tcc in range(1, TC):
                        scores_exp(tcc)
                        av_mm(tcc - 1)
                    av_mm(TC - 1)

                    # normalize: copy unnormalized; recip denom row (approx,
                    # ~2 ULP); K=1 matmul broadcasts the reciprocal; in-place
                    # multiply (SBUF x PSUM).
                    for hi2 in range(2):
                        av = avs[hi2]
                        rows = slice(DK * hi2, DK * (hi2 + 1))
                        nc.vector.tensor_copy(o2[rows, s0 : s0 + ST], av[0:DK, :])
                        dsb = sp.tile([1, ST], F32, name="dsb")
                        rsb = sp.tile([1, ST], F32, name="rsb")
                        scr = sp.tile([1, ST], F32, name="scr")
                        nc.scalar.copy(dsb[0:1, :], av[DK : DK + 1, :])
                        nc.vector.reciprocal_approx_accurate(
                            rsb[0:1, :], dsb[0:1, :], scr[0:1, :]
                        )
                        rb_ps = ps_sc.tile([P, ST], F32, name="sc")
                        for hf in range(2):
                            nc.tensor.matmul(
                                rb_ps[0:DK, hf * 512 : (hf + 1) * 512],
                                ones[0:1, :],
                                rsb[0:1, hf * 512 : (hf + 1) * 512],
                                start=True,
                                stop=True,
                            )
                        nc.vector.tensor_mul(
                            o2[rows, s0 : s0 + ST],
                            o2[rows, s0 : s0 + ST],
                            rb_ps[0:DK, :],
                        )

            # ---------------- Phase C: output projection (fp32) -------------
            for st7 in range(TC):
                of_ps = ps_av.tile([P, ST], F32, name="av")
                for c in range(2):
                    o2 = (o2a, o2b)[c]
                    for nh in range(2):
                        nc.tensor.matmul(
                            of_ps[:, nh * 512 : (nh + 1) * 512],
                            o2[:, st7 * P : (st7 + 1) * P],
                            wo_sb[:, c, nh * 512 : (nh + 1) * 512],
                            start=(c == 0),
                            stop=(c == 1),
                        )
                of = xp.tile([P, ST], F32, name="xc")
                nc.vector.tensor_copy(of[:], of_ps[:])
                nc.sync.dma_start(out[st7 * P : (st7 + 1) * P, :], of[:])

    nc.compile()
    return nc


def _get_nc():
    global _COMPILED
    if _COMPILED is None:
        _COMPILED = _build()
    return _COMPILED


def _split_bf16(x):
    import ml_dtypes

    hi = np.ascontiguousarray(x.astype(ml_dtypes.bfloat16))
    lo = np.ascontiguousarray(
        (x - hi.astype(np.float32)).astype(ml_dtypes.bfloat16)
    )
    return hi, lo


def _make_in_maps(q, k, v, w_q, b_q, w_k, b_k, w_v, b_v, w_o, b_o):
    q = np.asarray(q, np.float32)
    k = np.asarray(k, np.float32)
    v = np.asarray(v, np.float32)
    xs = {}
    for t, arr in (("q", q), ("k", k), ("v", v)):
        for b in range(2):
            xs[(t, b)] = _split_bf16(np.ascontiguousarray(arr[b].T))
    ws = {"q": np.asarray(w_q, np.float32), "k": np.asarray(w_k, np.float32),
          "v": np.asarray(w_v, np.float32)}
    bs = {"q": np.asarray(b_q, np.float32), "k": np.asarray(b_k, np.float32),
          "v": np.asarray(b_v, np.float32)}
    w_o = np.asarray(w_o, np.float32)
    in_maps = []
    for core in range(N_CORES):
        b, hg = divmod(core, 4)
        sl = slice(hg * DH, (hg + 1) * DH)
        m = {}
        for t in ("q", "k", "v"):
            m[f"x{t}h"], m[f"x{t}l"] = xs[(t, b)]
            wh, wl = _split_bf16(np.ascontiguousarray(ws[t][sl, :].T))
            m[f"w{t}h"], m[f"w{t}l"] = wh, wl
            m[f"b{t}"] = np.ascontiguousarray(bs[t][sl])
        m["wo"] = np.ascontiguousarray(w_o[:, sl].T)
        in_maps.append(m)
    return in_maps


def run(inputs, trace=False):
    from concourse.bass_utils import run_bass_kernel_spmd

    nc = _get_nc()
    in_maps = _make_in_maps(**inputs)
    res = run_bass_kernel_spmd(
        nc, in_maps, core_ids=list(range(N_CORES)), trace=trace
    )
    b_o = np.asarray(inputs["b_o"], np.float32)
    full = np.empty((2, S, DM), np.float32)
    for b in range(2):
        acc = res.results[4 * b]["out"].astype(np.float32)
        for hg in range(1, 4):
            acc = acc + res.results[4 * b + hg]["out"]
        full[b] = acc + b_o[None, :]
    return full, res


def kernel(**inputs) -> np.ndarray:
    full, _ = run(inputs, trace=False)
    return full
